# revision 9
# baseline (speedup 1.0000x reference)
"""Trainium2 Bass kernel for DecoderRNNTAtt (B=8, T=256, U=64, dims 512, odim 500).

Sharding: data-parallel over batch B across 8 cores (core i handles batch i).
Each core runs: attention-LSTM decoder scan (64 steps) + joint network.

Key tricks:
  - att_c @ W_a^T is folded as w @ (hs @ W_a^T)  (HW precomputed once)
  - embedding contribution ey_u @ W_e^T precomputed for all u (EYc), injected
    into the gate psum accumulation via a one-hot stationary matmul
  - all matmuls run as float32r (1 cycle/row at N>=256, near-fp32 precision)
  - joint computed transposed: out^T[odim, T] per u, so b_out is a
    per-partition ACT bias and W_out slices are stationary
"""

import os
import sys

sys.path.insert(0, "/opt/trn_rl_repo")

from contextlib import ExitStack

import numpy as np

from concourse import bacc, bass, mybir, tile
from concourse.bass_utils import run_bass_kernel_spmd

F32 = mybir.dt.float32
F32R = mybir.dt.float32r
AF = mybir.ActivationFunctionType
ALU = mybir.AluOpType
AX = mybir.AxisListType

B, T, U = 8, 256, 64
E = D = A = J = 512
G = 4 * D  # 2048
O = 500
OM = 125  # odim chunk (4 chunks of 125)
NCORES = 8

LAST_RESULTS = None
_CACHE = {}


# ----------------------------------------------------------------------------
# host-side packing helpers
# ----------------------------------------------------------------------------

def _pack_k(W):
    """[K, N] -> [128, K//128, N] with [p, c, n] = W[c*128+p, n]."""
    K, N = W.shape
    assert K % 128 == 0
    return np.ascontiguousarray(
        W.reshape(K // 128, 128, N).transpose(1, 0, 2)
    ).astype(np.float32)


def _pack_bias_cols(b, chunk=128):
    """[N] -> [128, N//chunk] col c rows 0..chunk-1 = b[c*chunk + p]."""
    n = b.shape[0]
    ncol = (n + chunk - 1) // chunk
    out = np.zeros((128, ncol), np.float32)
    for c in range(ncol):
        seg = b[c * chunk : (c + 1) * chunk]
        out[: seg.shape[0], c] = seg
    return out


def _prep_inputs(inputs):
    """Build per-core in_maps (host-side data layout only; no FLOPs besides
    the embedding gather)."""
    hs = np.asarray(inputs["hs_pad"], np.float32)          # [B, T, E]
    ys = np.asarray(inputs["ys_in_pad"])                   # [B, U] int
    hlens = np.asarray(inputs["hlens"]).astype(np.int64)   # [B]
    emb = np.asarray(inputs["emb"], np.float32)            # [O, E]

    W_ih0 = np.asarray(inputs["W_ih0"], np.float32)        # [G, E + E]
    W_hh0 = np.asarray(inputs["W_hh0"], np.float32)        # [G, D]
    b0 = (np.asarray(inputs["b_ih0"], np.float32)
          + np.asarray(inputs["b_hh0"], np.float32))       # [G]
    W_ih1 = np.asarray(inputs["W_ih1"], np.float32)        # [G, D]
    W_hh1 = np.asarray(inputs["W_hh1"], np.float32)        # [G, D]
    b1 = (np.asarray(inputs["b_ih1"], np.float32)
          + np.asarray(inputs["b_hh1"], np.float32))       # [G]

    W_ae = np.asarray(inputs["W_att_enc"], np.float32)     # [E, A]
    b_ae = np.asarray(inputs["b_att_enc"], np.float32)     # [A]
    W_ad = np.asarray(inputs["W_att_dec"], np.float32)     # [D, A]
    b_ad = np.asarray(inputs["b_att_dec"], np.float32)     # [A]
    W_le = np.asarray(inputs["W_lin_enc"], np.float32)     # [E, J]
    b_le = np.asarray(inputs["b_lin_enc"], np.float32)     # [J]
    W_ld = np.asarray(inputs["W_lin_dec"], np.float32)     # [D, J]
    W_out = np.asarray(inputs["W_out"], np.float32)        # [J, O]
    b_out = np.asarray(inputs["b_out"], np.float32)        # [O]

    W_e = W_ih0[:, :E]    # acts on ey
    W_a = W_ih0[:, E:]    # acts on att_c

    # shared (weight) tensors
    shared = {}
    shared["Wae"] = _pack_k(W_ae)                          # [128,4,A]
    shared["bae"] = _pack_bias_cols(b_ae)
    shared["Wad"] = _pack_k(W_ad)                          # [128,4,A]
    shared["bad"] = _pack_bias_cols(b_ad)
    shared["Wle"] = _pack_k(W_le)                          # [128,4,J]
    shared["ble"] = _pack_bias_cols(b_le)
    shared["WaT"] = _pack_k(np.ascontiguousarray(W_a.T))   # [128,4,G]
    WeT = _pack_k(np.ascontiguousarray(W_e.T))             # [128,4,G]
    aug = np.zeros((128, 1, G), np.float32)
    aug[0, 0, :] = b0
    shared["WeTg"] = np.concatenate([WeT, aug], axis=1)    # [128,5,G]
    shared["Whh0"] = _pack_k(np.ascontiguousarray(W_hh0.T))
    shared["Wih1"] = _pack_k(np.ascontiguousarray(W_ih1.T))
    shared["bih1"] = b1[None, :].astype(np.float32)        # [1,G]
    shared["Whh1"] = _pack_k(np.ascontiguousarray(W_hh1.T))
    shared["Wld"] = _pack_k(W_ld)                          # [128,4,J]
    shared["Wout"] = _pack_k(W_out)                        # [128,4,O]
    shared["bout"] = _pack_bias_cols(b_out, OM)            # [128,4]
    shared["I64"] = np.eye(64, dtype=np.float32)
    shared["one1"] = np.ones((1, 1), np.float32)
    shared["id1"] = np.ones((1, 1), np.float32)

    in_maps = []
    for b in range(NCORES):
        m = dict(shared)
        hsT = np.ascontiguousarray(hs[b].T)                # [E, T]
        m["hsT"] = _pack_k(hsT)                            # [128,4,T]
        ey = emb[ys[b]]                                    # [U, E] gather
        eyT = _pack_k(np.ascontiguousarray(ey.T))          # [128,4,U]
        aug = np.zeros((128, 1, U), np.float32)
        aug[0, 0, :] = 1.0
        m["EYTg"] = np.concatenate([eyT, aug], axis=1)     # [128,5,U]
        mneg = np.where(np.arange(T) < hlens[b], 0.0, -1e9)
        m["mneg"] = mneg[None, :].astype(np.float32)       # [1,T]
        in_maps.append(m)
    return in_maps


# ----------------------------------------------------------------------------
# kernel builder
# ----------------------------------------------------------------------------

def _build(n_steps=U):
    nc = bacc.Bacc(
        "TRN2", target_bir_lowering=False, debug=False, num_devices=NCORES
    )

    def din(name, shape, dt=F32):
        return nc.dram_tensor(name, list(shape), dt, kind="ExternalInput").ap()

    hsT_d = din("hsT", [128, 4, T], F32R)
    Wae_d = din("Wae", [128, 4, A], F32R)
    bae_d = din("bae", [128, 4])
    Wad_d = din("Wad", [128, 4, A])
    bad_d = din("bad", [128, 4])
    Wle_d = din("Wle", [128, 4, J], F32R)
    ble_d = din("ble", [128, 4])
    WaT_d = din("WaT", [128, 4, G], F32R)
    WeTg_d = din("WeTg", [128, 5, G], F32R)
    EYTg_d = din("EYTg", [128, 5, U], F32R)
    Whh0_d = din("Whh0", [128, 4, G], F32R)
    Wih1_d = din("Wih1", [128, 4, G], F32R)
    bih1_d = din("bih1", [1, G], F32R)
    Whh1_d = din("Whh1", [128, 4, G], F32R)
    Wld_d = din("Wld", [128, 4, J], F32R)
    Wout_d = din("Wout", [128, 4, O], F32R)
    bout_d = din("bout", [128, 4])
    mneg_d = din("mneg", [1, T])
    I64_d = din("I64", [64, 64], F32R)
    one1_d = din("one1", [1, 1], F32R)
    id1_d = din("id1", [1, 1])

    out_d = nc.dram_tensor(
        "out", [n_steps, 4, OM, T], F32, kind="ExternalOutput"
    ).ap()

    with tile.TileContext(nc) as tc, ExitStack() as ctx:
        # ---------------- persistent pool ----------------
        pers = ctx.enter_context(tc.tile_pool(name="pers", bufs=1))
        t_pre = pers.tile([128, 4, T], F32R, name="t_pre", tag="t_pre")
        t_aT = pers.tile([128, 4, T], F32, name="t_aT", tag="t_aT")
        t_HW = pers.tile([128, 2, G], F32R, name="t_HW", tag="t_HW")
        t_EYc = pers.tile([64, 4, 512], F32R, name="t_EYc", tag="t_EYc")
        t_Z1 = pers.tile([128, 4, U], F32R, name="t_Z1", tag="t_Z1")
        t_mneg = pers.tile([1, T], F32, name="t_mneg", tag="t_mneg")
        t_I64 = pers.tile([64, 64], F32R, name="t_I64", tag="t_I64")
        t_one1 = pers.tile([1, 1], F32R, name="t_one1", tag="t_one1")
        t_id1 = pers.tile([1, 1], F32, name="t_id1", tag="t_id1")
        t_Wad = pers.tile([128, 4, A], F32, name="t_Wad", tag="t_Wad")
        t_bad = pers.tile([128, 4], F32, name="t_bad", tag="t_bad")
        t_c0 = pers.tile([1, D], F32, name="t_c0", tag="t_c0")
        t_c1 = pers.tile([1, D], F32, name="t_c1", tag="t_c1")
        t_zi = pers.tile([128, 4], F32R, name="t_zi", tag="t_zi")

        nc.sync.dma_start(t_mneg[:], mneg_d[:])
        nc.sync.dma_start(t_I64[:], I64_d[:])
        nc.sync.dma_start(t_one1[:], one1_d[:])
        nc.sync.dma_start(t_id1[:], id1_d[:])
        nc.sync.dma_start(t_Wad[:], Wad_d[:])
        nc.sync.dma_start(t_bad[:], bad_d[:])
        nc.vector.memset(t_c0[:], 0.0)
        nc.vector.memset(t_c1[:], 0.0)
        t_zf = pers.tile([128, 4, U], F32, name="t_zf", tag="t_zf")
        nc.vector.memset(t_zf[:], 0.0)
        nc.vector.tensor_copy(t_zi[:], t_zf[:, :, 0])
        t_zif = pers.tile([128, 4], F32, name="t_zif", tag="t_zif")
        nc.vector.memset(t_zif[:], 0.0)
        nc.vector.tensor_copy(t_Z1[:], t_zf[:])

        # scratch pool (small per-step tiles)
        scr = ctx.enter_context(tc.tile_pool(name="scr", bufs=1))

        # psum pool for phases A+B: tags pe(2) + small(2) + g(1x4banks)
        psAB = ctx.enter_context(ExitStack())
        psA = psAB.enter_context(tc.tile_pool(name="psA", bufs=1, space="PSUM"))

        # ---------------- phase A ----------------
        with tc.tile_pool(name="phA", bufs=1) as pA:
            t_hsT = pA.tile([128, 4, T], F32R, name="t_hsT", tag="t_hsT")
            t_Wae = pA.tile([128, 4, A], F32R, name="t_Wae", tag="t_Wae")
            t_bae = pA.tile([128, 4], F32, name="t_bae", tag="t_bae")
            t_Wle = pA.tile([128, 4, J], F32R, name="t_Wle", tag="t_Wle")
            t_ble = pA.tile([128, 4], F32, name="t_ble", tag="t_ble")
            t_WaT = pA.tile([128, 4, G], F32R, name="t_WaT", tag="t_WaT")
            t_WeTg = pA.tile([128, 5, G], F32R, name="t_WeTg", tag="t_WeTg")
            t_EYTg = pA.tile([128, 5, U], F32R, name="t_EYTg", tag="t_EYTg")

            nc.sync.dma_start(t_hsT[:], hsT_d[:])
            nc.sync.dma_start(t_Wae[:], Wae_d[:])
            nc.sync.dma_start(t_bae[:], bae_d[:])
            nc.sync.dma_start(t_Wle[:], Wle_d[:])
            nc.sync.dma_start(t_ble[:], ble_d[:])
            nc.sync.dma_start(t_WaT[:], WaT_d[:])
            nc.sync.dma_start(t_WeTg[:], WeTg_d[:])
            nc.sync.dma_start(t_EYTg[:], EYTg_d[:])

            # pre_enc[a, t] = tanh(sum_e hs[t,e] Wae[e,a] + bae[a])
            for ca in range(4):
                pe_ps = psA.tile([128, T], F32, name="pe_ps", tag="pe", bufs=2)
                for ce in range(4):
                    nc.tensor.matmul(
                        pe_ps[:],
                        (t_Wae[:, ce, ca * 128 : (ca + 1) * 128]),
                        (t_hsT[:, ce, :]),
                        start=(ce == 0),
                        stop=(ce == 3),
                    )
                nc.scalar.activation(
                    t_pre[:, ca, :], pe_ps[:], AF.Tanh,
                    bias=t_bae[:, ca : ca + 1],
                )

            # aT[j, t] = sum_e hs[t,e] Wle[e,j] + ble[j]
            for cj in range(4):
                a_ps = psA.tile([128, T], F32, name="a_ps", tag="pe", bufs=2)
                for ce in range(4):
                    nc.tensor.matmul(
                        a_ps[:],
                        (t_Wle[:, ce, cj * 128 : (cj + 1) * 128]),
                        (t_hsT[:, ce, :]),
                        start=(ce == 0),
                        stop=(ce == 3),
                    )
                nc.scalar.activation(
                    t_aT[:, cj, :], a_ps[:], AF.Identity,
                    bias=t_ble[:, cj : cj + 1],
                )

            # HW[t, g] = sum_e hs[t,e] WaT[e,g]
            for ct in range(2):
                for jg in range(4):
                    hw_ps = psA.tile([128, 512], F32, name="hw_ps", tag="pe", bufs=2)
                    for ce in range(4):
                        nc.tensor.matmul(
                            hw_ps[:],
                            (t_hsT[:, ce, ct * 128 : (ct + 1) * 128]),
                            (t_WaT[:, ce, jg * 512 : (jg + 1) * 512]),
                            start=(ce == 0),
                            stop=(ce == 3),
                        )
                    nc.vector.tensor_copy(
                        t_HW[:, ct, jg * 512 : (jg + 1) * 512], hw_ps[:]
                    )

            # EYc[u, g] = sum_e ey[u,e] WeT[e,g]  (+ b0 via aug chunk 4)
            for jg in range(4):
                ey_ps = psA.tile([64, 512], F32, name="ey_ps", tag="pe", bufs=2)
                for ce in range(5):
                    nc.tensor.matmul(
                        ey_ps[:],
                        (t_EYTg[:, ce, :]),
                        (t_WeTg[:, ce, jg * 512 : (jg + 1) * 512]),
                        start=(ce == 0),
                        stop=(ce == 4),
                    )
                nc.vector.tensor_copy(t_EYc[:, jg, :], ey_ps[:])

        # ---------------- phase B: decoder scan ----------------
        with tc.tile_pool(name="phB", bufs=1) as pB:
            t_Whh0 = pB.tile([128, 4, G], F32R, name="t_Whh0", tag="t_Whh0")
            t_Wih1 = pB.tile([128, 4, G], F32R, name="t_Wih1", tag="t_Wih1")
            t_bih1 = pB.tile([1, G], F32R, name="t_bih1", tag="t_bih1")
            t_Whh1 = pB.tile([128, 4, G], F32R, name="t_Whh1", tag="t_Whh1")
            nc.sync.dma_start(t_Whh0[:], Whh0_d[:])
            nc.sync.dma_start(t_Wih1[:], Wih1_d[:])
            nc.sync.dma_start(t_bih1[:], bih1_d[:])
            nc.sync.dma_start(t_Whh1[:], Whh1_d[:])

            z0T_prev = t_zi
            z0Tf_prev = t_zif
            for u in range(n_steps):
                z1T_prev = t_zi if u == 0 else t_Z1[:, :, u - 1]

                # qT[a] = tanh(sum_d Wad[d,a] z0[d] + bad[a])
                psq = psA.tile([128, 4], F32, name="psq", tag="small", bufs=2)
                for ca in range(4):
                    for cd in range(4):
                        nc.tensor.matmul(
                            psq[:, ca : ca + 1],
                            (t_Wad[:, cd, ca * 128 : (ca + 1) * 128]),
                            (z0Tf_prev[:, cd : cd + 1]),
                            start=(cd == 0),
                            stop=(cd == 3),
                        )
                qT = scr.tile([128, 4], F32R, name="qT", tag="qT", bufs=2)
                for ca in range(4):
                    nc.scalar.activation(
                        qT[:, ca : ca + 1], psq[:, ca : ca + 1], AF.Tanh,
                        bias=t_bad[:, ca : ca + 1],
                    )

                # e[t] = sum_a pre_enc[a,t] q[a]   (+ mask)
                eps = psA.tile([1, T], F32, name="eps", tag="small", bufs=2)
                for ca in range(4):
                    nc.tensor.matmul(
                        eps[:],
                        (qT[:, ca : ca + 1]),
                        (t_pre[:, ca, :]),
                        start=(ca == 0),
                        stop=(ca == 3),
                    )
                em = scr.tile([1, T], F32, name="em", tag="em")
                nc.vector.tensor_tensor(em[:], eps[:], t_mneg[:], ALU.add)

                # softmax row
                ngmx = scr.tile([1, 1], F32, name="ngmx", tag="ngmx")
                nc.vector.tensor_reduce(
                    ngmx[:], em[:], AX.X, ALU.max, negate=True
                )
                wsc = scr.tile([1, T], F32, name="wsc", tag="wsc")
                sume = scr.tile([1, 1], F32, name="sume", tag="sume")
                nc.scalar.activation(
                    wsc[:], em[:], AF.Exp, bias=ngmx[:], accum_out=sume[:]
                )
                rinv = scr.tile([1, 1], F32, name="rinv", tag="rinv")
                nc.vector.reciprocal(rinv[:], sume[:])
                wrow = scr.tile([1, T], F32, name="wrow", tag="wrow")
                nc.vector.tensor_scalar_mul(wrow[:], wsc[:], rinv[:])

                # wT columns
                wps = psA.tile([128, 2], F32, name="wps", tag="small", bufs=2)
                for ct in range(2):
                    nc.tensor.transpose(
                        wps[:, ct : ct + 1],
                        wrow[0:1, ct * 128 : (ct + 1) * 128],
                        t_id1[:],
                    )
                wT = scr.tile([128, 2], F32R, name="wT", tag="wT", bufs=2)
                nc.vector.tensor_copy(wT[:], wps[:])

                # gates0 = w@HW + EYc[u] + Whh0@z0
                g0 = psA.tile([1, 4, 512], F32, name="g0", tag="g", bufs=1)
                for jg in range(4):
                    sl = slice(jg * 512, (jg + 1) * 512)
                    for ct in range(2):
                        nc.tensor.matmul(
                            g0[:, jg, :],
                            (wT[:, ct : ct + 1]),
                            (t_HW[:, ct, sl]),
                            start=(ct == 0),
                            stop=False,
                        )
                    nc.tensor.matmul(
                        g0[:, jg, :],
                        (t_I64[:, u : u + 1]),
                        (t_EYc[:, jg, :]),
                        start=False,
                        stop=False,
                    )
                    for cd in range(4):
                        nc.tensor.matmul(
                            g0[:, jg, :],
                            (z0T_prev[:, cd : cd + 1]),
                            (t_Whh0[:, cd, sl]),
                            start=False,
                            stop=(cd == 3),
                        )

                # pointwise cell 0  (gate order i, f, g, o)
                si = scr.tile([1, D], F32, name="si", tag="si")
                sf = scr.tile([1, D], F32, name="sf", tag="sf")
                tg = scr.tile([1, D], F32, name="tg", tag="tg")
                so = scr.tile([1, D], F32, name="so", tag="so")
                nc.scalar.activation(si[:], g0[:, 0, :], AF.Sigmoid)
                nc.scalar.activation(sf[:], g0[:, 1, :], AF.Sigmoid)
                nc.scalar.activation(tg[:], g0[:, 2, :], AF.Tanh)
                nc.scalar.activation(so[:], g0[:, 3, :], AF.Sigmoid)
                tt1 = scr.tile([1, D], F32, name="tt1", tag="tt1")
                tt2 = scr.tile([1, D], F32, name="tt2", tag="tt2")
                nc.vector.tensor_tensor(tt1[:], sf[:], t_c0[:], ALU.mult)
                nc.vector.tensor_tensor(tt2[:], si[:], tg[:], ALU.mult)
                nc.vector.tensor_tensor(t_c0[:], tt1[:], tt2[:], ALU.add)
                th0 = scr.tile([1, D], F32, name="th0", tag="th0")
                nc.scalar.activation(th0[:], t_c0[:], AF.Tanh)
                z0r = scr.tile([1, D], F32, name="z0r", tag="z0r")
                nc.vector.tensor_tensor(z0r[:], so[:], th0[:], ALU.mult)

                # z0T
                zps = psA.tile([128, 4], F32, name="zps", tag="small", bufs=2)
                for cd in range(4):
                    nc.tensor.transpose(
                        zps[:, cd : cd + 1],
                        z0r[0:1, cd * 128 : (cd + 1) * 128],
                        t_id1[:],
                    )
                z0T = scr.tile([128, 4], F32R, name="z0T", tag="z0T", bufs=2)
                nc.vector.tensor_copy(z0T[:], zps[:])
                z0Tf = scr.tile([128, 4], F32, name="z0Tf", tag="z0Tf", bufs=2)
                nc.vector.tensor_copy(z0Tf[:], zps[:])

                # gates1 = Wih1@z0new + bih1 + Whh1@z1
                g1 = psA.tile([1, 4, 512], F32, name="g1", tag="g", bufs=1)
                for jg in range(4):
                    sl = slice(jg * 512, (jg + 1) * 512)
                    for cd in range(4):
                        nc.tensor.matmul(
                            g1[:, jg, :],
                            (z1T_prev[:, cd : cd + 1]),
                            (t_Whh1[:, cd, sl]),
                            start=(cd == 0),
                            stop=False,
                        )
                    nc.tensor.matmul(
                        g1[:, jg, :],
                        (t_one1[:]),
                        (t_bih1[:, sl]),
                        start=False,
                        stop=False,
                    )
                    for cd in range(4):
                        nc.tensor.matmul(
                            g1[:, jg, :],
                            (z0T[:, cd : cd + 1]),
                            (t_Wih1[:, cd, sl]),
                            start=False,
                            stop=(cd == 3),
                        )

                # pointwise cell 1
                si1 = scr.tile([1, D], F32, name="si1", tag="si")
                sf1 = scr.tile([1, D], F32, name="sf1", tag="sf")
                tg1 = scr.tile([1, D], F32, name="tg1", tag="tg")
                so1 = scr.tile([1, D], F32, name="so1", tag="so")
                nc.scalar.activation(si1[:], g1[:, 0, :], AF.Sigmoid)
                nc.scalar.activation(sf1[:], g1[:, 1, :], AF.Sigmoid)
                nc.scalar.activation(tg1[:], g1[:, 2, :], AF.Tanh)
                nc.scalar.activation(so1[:], g1[:, 3, :], AF.Sigmoid)
                tt3 = scr.tile([1, D], F32, name="tt3", tag="tt1")
                tt4 = scr.tile([1, D], F32, name="tt4", tag="tt2")
                nc.vector.tensor_tensor(tt3[:], sf1[:], t_c1[:], ALU.mult)
                nc.vector.tensor_tensor(tt4[:], si1[:], tg1[:], ALU.mult)
                nc.vector.tensor_tensor(t_c1[:], tt3[:], tt4[:], ALU.add)
                th1 = scr.tile([1, D], F32, name="th1", tag="th0")
                nc.scalar.activation(th1[:], t_c1[:], AF.Tanh)
                z1r = scr.tile([1, D], F32, name="z1r", tag="z0r")
                nc.vector.tensor_tensor(z1r[:], so1[:], th1[:], ALU.mult)

                # z1T -> Z1 column u
                zps1 = psA.tile([128, 4, 1], F32, name="zps1", tag="small", bufs=2)
                for cd in range(4):
                    nc.tensor.transpose(
                        zps1[:, cd, :],
                        z1r[0:1, cd * 128 : (cd + 1) * 128],
                        t_id1[:],
                    )
                nc.vector.tensor_copy(t_Z1[:, :, u : u + 1], zps1[:])

                z0T_prev = z0T
                z0Tf_prev = z0Tf

        # ---------------- phase C: joint ----------------
        psAB.close()
        with tc.tile_pool(name="phC", bufs=1) as pC, \
             tc.tile_pool(name="psC", bufs=1, space="PSUM") as psC:
            t_Wld = pC.tile([128, 4, J], F32R, name="t_Wld", tag="t_Wld")
            t_Wout = pC.tile([128, 4, O], F32R, name="t_Wout", tag="t_Wout")
            t_bout = pC.tile([128, 4], F32, name="t_bout", tag="t_bout")
            nc.sync.dma_start(t_Wld[:], Wld_d[:])
            nc.sync.dma_start(t_Wout[:], Wout_d[:])
            nc.sync.dma_start(t_bout[:], bout_d[:])

            # DT[j, u] = sum_d Wld[d,j] z1[u,d]
            t_DT = pC.tile([128, 4, U], F32, name="t_DT", tag="t_DT")
            for cj in range(4):
                dps = psC.tile([128, U], F32, name="dps", tag="j", bufs=4)
                for cd in range(4):
                    nc.tensor.matmul(
                        dps[:],
                        (t_Wld[:, cd, cj * 128 : (cj + 1) * 128]),
                        (t_Z1[:, cd, :]),
                        start=(cd == 0),
                        stop=(cd == 3),
                    )
                nc.vector.tensor_copy(t_DT[:, cj, :], dps[:])

            for u in range(n_steps):
                zt = pC.tile([128, 4, T], F32, name="zt", tag="zt", bufs=2)
                for cj in range(4):
                    nc.vector.tensor_scalar(
                        zt[:, cj, :], t_aT[:, cj, :],
                        t_DT[:, cj, u : u + 1], None, ALU.add,
                    )
                zth = pC.tile([128, 4, T], F32R, name="zth", tag="zth", bufs=2)
                nc.scalar.activation(zth[:], zt[:], AF.Tanh)

                outU = pC.tile([OM, 4, T], F32, name="outU", tag="outU", bufs=3)
                for m in range(4):
                    pj = psC.tile([OM, T], F32, name="pj", tag="j", bufs=4)
                    for cj in range(4):
                        nc.tensor.matmul(
                            pj[:],
                            (t_Wout[:, cj, m * OM : (m + 1) * OM]),
                            (zth[:, cj, :]),
                            start=(cj == 0),
                            stop=(cj == 3),
                        )
                    nc.scalar.activation(
                        outU[:, m, :], pj[:], AF.Identity,
                        bias=t_bout[0:OM, m : m + 1],
                    )
                    nc.sync.dma_start(
                        out_d[u : u + 1, m : m + 1, :, :], outU[:, m : m + 1, :]
                    )

    nc.compile()
    return nc


# ----------------------------------------------------------------------------
# entry point
# ----------------------------------------------------------------------------

def kernel(**inputs):
    global LAST_RESULTS
    if "nc" not in _CACHE:
        _CACHE["nc"] = _build(U)
    nc = _CACHE["nc"]
    in_maps = _prep_inputs(inputs)
    res = run_bass_kernel_spmd(
        nc, in_maps, list(range(NCORES)),
        trace=bool(int(os.environ.get("KBENCH_TRACE", "0"))),
    )
    LAST_RESULTS = res
    outs = []
    for c in range(NCORES):
        o = res.results[c]["out"]              # [U, 4, 125, T]
        o = o.reshape(U, O, T).transpose(2, 0, 1)  # [T, U, O]
        outs.append(np.ascontiguousarray(o))
    full = np.stack(outs, axis=0).astype(np.float32)  # [B, T, U, O]
    return full


# revision 12
# speedup vs baseline: 1.4865x; 1.4865x over previous
"""Trainium2 Bass kernel for DecoderRNNTAtt (B=8, T=256, U=64, dims 512, odim 500).

Sharding: data-parallel over batch B across 8 cores (core i handles batch i).
Each core runs: attention-LSTM decoder scan (64 steps) + joint network.

Key tricks:
  - att_c @ W_a^T is folded as w @ (hs @ W_a^T)  (HW precomputed once)
  - embedding contribution ey_u @ W_e^T precomputed for all u (EYc), injected
    into the gate psum accumulation via a one-hot stationary matmul
  - all matmuls run as float32r (1 cycle/row at N>=256, near-fp32 precision)
  - joint computed transposed: out^T[odim, T] per u, so b_out is a
    per-partition ACT bias and W_out slices are stationary
"""

import os
import sys

sys.path.insert(0, "/opt/trn_rl_repo")

from contextlib import ExitStack

import numpy as np

from concourse import bacc, bass, mybir, tile
from concourse.bass_utils import run_bass_kernel_spmd

F32 = mybir.dt.float32
F32R = mybir.dt.float32r
AF = mybir.ActivationFunctionType
ALU = mybir.AluOpType
AX = mybir.AxisListType

B, T, U = 8, 256, 64
E = D = A = J = 512
G = 4 * D  # 2048
O = 500
OM = 125  # odim chunk (4 chunks of 125)
NCORES = 8

LAST_RESULTS = None
_CACHE = {}


# ----------------------------------------------------------------------------
# host-side packing helpers
# ----------------------------------------------------------------------------

def _pack_k(W):
    """[K, N] -> [128, K//128, N] with [p, c, n] = W[c*128+p, n]."""
    K, N = W.shape
    assert K % 128 == 0
    return np.ascontiguousarray(
        W.reshape(K // 128, 128, N).transpose(1, 0, 2)
    ).astype(np.float32)


def _pack_bias_cols(b, chunk=128):
    """[N] -> [128, N//chunk] col c rows 0..chunk-1 = b[c*chunk + p]."""
    n = b.shape[0]
    ncol = (n + chunk - 1) // chunk
    out = np.zeros((128, ncol), np.float32)
    for c in range(ncol):
        seg = b[c * chunk : (c + 1) * chunk]
        out[: seg.shape[0], c] = seg
    return out


def _prep_inputs(inputs):
    """Build per-core in_maps (host-side data layout only; no FLOPs besides
    the embedding gather)."""
    hs = np.asarray(inputs["hs_pad"], np.float32)          # [B, T, E]
    ys = np.asarray(inputs["ys_in_pad"])                   # [B, U] int
    hlens = np.asarray(inputs["hlens"]).astype(np.int64)   # [B]
    emb = np.asarray(inputs["emb"], np.float32)            # [O, E]

    W_ih0 = np.asarray(inputs["W_ih0"], np.float32)        # [G, E + E]
    W_hh0 = np.asarray(inputs["W_hh0"], np.float32)        # [G, D]
    b0 = (np.asarray(inputs["b_ih0"], np.float32)
          + np.asarray(inputs["b_hh0"], np.float32))       # [G]
    W_ih1 = np.asarray(inputs["W_ih1"], np.float32)        # [G, D]
    W_hh1 = np.asarray(inputs["W_hh1"], np.float32)        # [G, D]
    b1 = (np.asarray(inputs["b_ih1"], np.float32)
          + np.asarray(inputs["b_hh1"], np.float32))       # [G]

    W_ae = np.asarray(inputs["W_att_enc"], np.float32)     # [E, A]
    b_ae = np.asarray(inputs["b_att_enc"], np.float32)     # [A]
    W_ad = np.asarray(inputs["W_att_dec"], np.float32)     # [D, A]
    b_ad = np.asarray(inputs["b_att_dec"], np.float32)     # [A]
    W_le = np.asarray(inputs["W_lin_enc"], np.float32)     # [E, J]
    b_le = np.asarray(inputs["b_lin_enc"], np.float32)     # [J]
    W_ld = np.asarray(inputs["W_lin_dec"], np.float32)     # [D, J]
    W_out = np.asarray(inputs["W_out"], np.float32)        # [J, O]
    b_out = np.asarray(inputs["b_out"], np.float32)        # [O]

    W_e = W_ih0[:, :E]    # acts on ey
    W_a = W_ih0[:, E:]    # acts on att_c

    # Device convention: hidden states are stored DOUBLED (Z = 2z) because
    # sigmoid/tanh are computed as tanh with ACT-side input scaling.  All
    # weights that consume a hidden state are pre-halved here.
    # shared (weight) tensors
    shared = {}
    shared["Wae"] = _pack_k(W_ae)                          # [128,4,A]
    shared["bae"] = _pack_bias_cols(b_ae)
    shared["Wad"] = _pack_k(0.5 * W_ad)                    # [128,4,A]
    shared["badr"] = b_ad[None, :].astype(np.float32)      # [1,A]
    shared["Wle"] = _pack_k(W_le)                          # [128,4,J]
    shared["ble"] = _pack_bias_cols(b_le)
    shared["WaT"] = _pack_k(np.ascontiguousarray(W_a.T))   # [128,4,G]
    WeT = _pack_k(np.ascontiguousarray(W_e.T))             # [128,4,G]
    aug = np.zeros((128, 1, G), np.float32)
    aug[0, 0, :] = b0
    shared["WeTg"] = np.concatenate([WeT, aug], axis=1)    # [128,5,G]
    shared["Whh0"] = _pack_k(np.ascontiguousarray(0.5 * W_hh0.T))
    shared["Wih1"] = _pack_k(np.ascontiguousarray(0.5 * W_ih1.T))
    b1s = b1.copy()        # pre-scaled bias row for the fused DVE add
    b1s[:D] *= 0.5         # i
    b1s[D:2*D] *= 0.5      # f
    b1s[3*D:] *= 0.5       # o  (g stays 1.0)
    shared["bih1"] = b1s[None, :].astype(np.float32)       # [1,G]
    shared["Whh1"] = _pack_k(np.ascontiguousarray(0.5 * W_hh1.T))
    shared["Wld"] = _pack_k(0.5 * W_ld)                    # [128,4,J]
    shared["Wout"] = _pack_k(W_out)                        # [128,4,O]
    shared["bout"] = _pack_bias_cols(b_out, OM)            # [128,4]
    shared["I64"] = np.eye(64, dtype=np.float32)
    shared["one1"] = np.ones((1, 1), np.float32)
    shared["id1"] = np.ones((1, 1), np.float32)

    in_maps = []
    for b in range(NCORES):
        m = dict(shared)
        hsT = np.ascontiguousarray(hs[b].T)                # [E, T]
        m["hsT"] = _pack_k(hsT)                            # [128,4,T]
        ey = emb[ys[b]]                                    # [U, E] gather
        eyT = _pack_k(np.ascontiguousarray(ey.T))          # [128,4,U]
        aug = np.zeros((128, 1, U), np.float32)
        aug[0, 0, :] = 1.0
        m["EYTg"] = np.concatenate([eyT, aug], axis=1)     # [128,5,U]
        mneg = np.where(np.arange(T) < hlens[b], 0.0, -1e9)
        m["mneg"] = mneg[None, :].astype(np.float32)       # [1,T]
        in_maps.append(m)
    return in_maps


# ----------------------------------------------------------------------------
# kernel builder
# ----------------------------------------------------------------------------

def _build(n_steps=U):
    nc = bacc.Bacc(
        "TRN2", target_bir_lowering=False, debug=False, num_devices=NCORES
    )

    def din(name, shape, dt=F32):
        return nc.dram_tensor(name, list(shape), dt, kind="ExternalInput").ap()

    hsT_d = din("hsT", [128, 4, T], F32R)
    Wae_d = din("Wae", [128, 4, A], F32R)
    bae_d = din("bae", [128, 4])
    Wad_d = din("Wad", [128, 4, A], F32R)
    badr_d = din("badr", [1, A], F32R)
    Wle_d = din("Wle", [128, 4, J], F32R)
    ble_d = din("ble", [128, 4])
    WaT_d = din("WaT", [128, 4, G], F32R)
    WeTg_d = din("WeTg", [128, 5, G], F32R)
    EYTg_d = din("EYTg", [128, 5, U], F32R)
    Whh0_d = din("Whh0", [128, 4, G], F32R)
    Wih1_d = din("Wih1", [128, 4, G], F32R)
    bih1_d = din("bih1", [1, G])
    Whh1_d = din("Whh1", [128, 4, G], F32R)
    Wld_d = din("Wld", [128, 4, J], F32R)
    Wout_d = din("Wout", [128, 4, O], F32R)
    bout_d = din("bout", [128, 4])
    mneg_d = din("mneg", [1, T])
    I64_d = din("I64", [64, 64], F32R)
    one1_d = din("one1", [1, 1], F32R)
    id1_d = din("id1", [1, 1])

    out_d = nc.dram_tensor(
        "out", [4, OM, n_steps, T], F32, kind="ExternalOutput"
    ).ap()

    SC = [0.5, 0.5, 1.0, 0.5]  # tanh input scale per gate (i, f, g, o)

    with tile.TileContext(nc) as tc, ExitStack() as ctx:
        # ---------------- persistent pool ----------------
        pers = ctx.enter_context(tc.tile_pool(name="pers", bufs=1))
        t_pre = pers.tile([128, 4, T], F32R, name="t_pre", tag="t_pre")
        t_aT = pers.tile([128, 4, T], F32, name="t_aT", tag="t_aT")
        t_HW = pers.tile([128, 2, G], F32R, name="t_HW", tag="t_HW")
        t_EYc = pers.tile([64, 4, 512], F32R, name="t_EYc", tag="t_EYc")
        t_Z1 = pers.tile([128, 4, U], F32R, name="t_Z1", tag="t_Z1")
        t_mneg = pers.tile([1, T], F32, name="t_mneg", tag="t_mneg")
        t_I64 = pers.tile([64, 64], F32R, name="t_I64", tag="t_I64")
        t_one1 = pers.tile([1, 1], F32R, name="t_one1", tag="t_one1")
        t_id1 = pers.tile([1, 1], F32, name="t_id1", tag="t_id1")
        t_Wad = pers.tile([128, 4, A], F32R, name="t_Wad", tag="t_Wad")
        t_badr = pers.tile([1, A], F32R, name="t_badr", tag="t_badr")
        t_c0 = pers.tile([1, D], F32, name="t_c0", tag="t_c0")
        t_c1 = pers.tile([1, D], F32, name="t_c1", tag="t_c1")
        t_zi = pers.tile([128, 4], F32R, name="t_zi", tag="t_zi")

        nc.sync.dma_start(t_mneg[:], mneg_d[:])
        nc.sync.dma_start(t_I64[:], I64_d[:])
        nc.sync.dma_start(t_one1[:], one1_d[:])
        nc.sync.dma_start(t_id1[:], id1_d[:])
        nc.sync.dma_start(t_Wad[:], Wad_d[:])
        nc.sync.dma_start(t_badr[:], badr_d[:])
        nc.vector.memset(t_c0[:], 0.0)
        nc.vector.memset(t_c1[:], 0.0)
        t_zf = pers.tile([128, 4, U], F32, name="t_zf", tag="t_zf")
        nc.vector.memset(t_zf[:], 0.0)
        nc.vector.tensor_copy(t_zi[:], t_zf[:, :, 0])
        nc.vector.tensor_copy(t_Z1[:], t_zf[:])

        # scratch pool (small per-step tiles)
        scr = ctx.enter_context(tc.tile_pool(name="scr", bufs=1))

        # psum pool for phases A+B: pe(2) + small(2) + g(1x4 banks)
        psAB = ctx.enter_context(ExitStack())
        psA = psAB.enter_context(tc.tile_pool(name="psA", bufs=1, space="PSUM"))

        # ---------------- phase A ----------------
        with tc.tile_pool(name="phA", bufs=1) as pA:
            t_hsT = pA.tile([128, 4, T], F32R, name="t_hsT", tag="t_hsT")
            t_Wae = pA.tile([128, 4, A], F32R, name="t_Wae", tag="t_Wae")
            t_bae = pA.tile([128, 4], F32, name="t_bae", tag="t_bae")
            t_Wle = pA.tile([128, 4, J], F32R, name="t_Wle", tag="t_Wle")
            t_ble = pA.tile([128, 4], F32, name="t_ble", tag="t_ble")
            t_WaT = pA.tile([128, 4, G], F32R, name="t_WaT", tag="t_WaT")
            t_WeTg = pA.tile([128, 5, G], F32R, name="t_WeTg", tag="t_WeTg")
            t_EYTg = pA.tile([128, 5, U], F32R, name="t_EYTg", tag="t_EYTg")

            nc.sync.dma_start(t_hsT[:], hsT_d[:])
            nc.sync.dma_start(t_Wae[:], Wae_d[:])
            nc.sync.dma_start(t_bae[:], bae_d[:])
            nc.sync.dma_start(t_Wle[:], Wle_d[:])
            nc.sync.dma_start(t_ble[:], ble_d[:])
            nc.sync.dma_start(t_WaT[:], WaT_d[:])
            nc.sync.dma_start(t_WeTg[:], WeTg_d[:])
            nc.sync.dma_start(t_EYTg[:], EYTg_d[:])

            # pre_enc[a, t] = tanh(sum_e hs[t,e] Wae[e,a] + bae[a])
            for ca in range(4):
                pe_ps = psA.tile([128, T], F32, name="pe_ps", tag="pe", bufs=2)
                for ce in range(4):
                    nc.tensor.matmul(
                        pe_ps[:],
                        t_Wae[:, ce, ca * 128 : (ca + 1) * 128],
                        t_hsT[:, ce, :],
                        start=(ce == 0),
                        stop=(ce == 3),
                    )
                nc.scalar.activation(
                    t_pre[:, ca, :], pe_ps[:], AF.Tanh,
                    bias=t_bae[:, ca : ca + 1],
                )

            # aT[j, t] = sum_e hs[t,e] Wle[e,j] + ble[j]
            for cj in range(4):
                a_ps = psA.tile([128, T], F32, name="a_ps", tag="pe", bufs=2)
                for ce in range(4):
                    nc.tensor.matmul(
                        a_ps[:],
                        t_Wle[:, ce, cj * 128 : (cj + 1) * 128],
                        t_hsT[:, ce, :],
                        start=(ce == 0),
                        stop=(ce == 3),
                    )
                nc.scalar.activation(
                    t_aT[:, cj, :], a_ps[:], AF.Identity,
                    bias=t_ble[:, cj : cj + 1],
                )

            # HW[t, g] = sum_e hs[t,e] WaT[e,g]
            for ct in range(2):
                for jg in range(4):
                    hw_ps = psA.tile([128, 512], F32, name="hw_ps", tag="pe", bufs=2)
                    for ce in range(4):
                        nc.tensor.matmul(
                            hw_ps[:],
                            t_hsT[:, ce, ct * 128 : (ct + 1) * 128],
                            t_WaT[:, ce, jg * 512 : (jg + 1) * 512],
                            start=(ce == 0),
                            stop=(ce == 3),
                        )
                    nc.vector.tensor_copy(
                        t_HW[:, ct, jg * 512 : (jg + 1) * 512], hw_ps[:]
                    )

            # EYc[u, g] = sum_e ey[u,e] WeT[e,g]  (+ b0 via aug chunk 4)
            for jg in range(4):
                ey_ps = psA.tile([64, 512], F32, name="ey_ps", tag="pe", bufs=2)
                for ce in range(5):
                    nc.tensor.matmul(
                        ey_ps[:],
                        t_EYTg[:, ce, :],
                        t_WeTg[:, ce, jg * 512 : (jg + 1) * 512],
                        start=(ce == 0),
                        stop=(ce == 4),
                    )
                nc.vector.tensor_copy(t_EYc[:, jg, :], ey_ps[:])

        # ---------------- phase B: decoder scan ----------------
        with tc.tile_pool(name="phB", bufs=1) as pB:
            t_Whh0 = pB.tile([128, 4, G], F32R, name="t_Whh0", tag="t_Whh0")
            t_Wih1 = pB.tile([128, 4, G], F32R, name="t_Wih1", tag="t_Wih1")
            t_bih1 = pB.tile([1, G], F32, name="t_bih1", tag="t_bih1")
            t_Whh1 = pB.tile([128, 4, G], F32R, name="t_Whh1", tag="t_Whh1")
            nc.sync.dma_start(t_Whh0[:], Whh0_d[:])
            nc.sync.dma_start(t_Wih1[:], Wih1_d[:])
            nc.sync.dma_start(t_bih1[:], bih1_d[:])
            nc.sync.dma_start(t_Whh1[:], Whh1_d[:])

            z0T_prev = t_zi
            for u in range(n_steps):
                z1T_prev = t_zi if u == 0 else t_Z1[:, :, u - 1]

                # q row = tanh(Z0 @ (Wad/2) + b_ad)
                qrow = psA.tile([1, A], F32, name="qrow", tag="pe", bufs=2)
                for cd in range(4):
                    nc.tensor.matmul(
                        qrow[:],
                        z0T_prev[:, cd : cd + 1],
                        t_Wad[:, cd, :],
                        start=(cd == 0),
                        stop=False,
                    )
                nc.tensor.matmul(
                    qrow[:], t_one1[:], t_badr[:], start=False, stop=True
                )
                qrs = scr.tile([1, A], F32, name="qrs", tag="qrs")
                nc.scalar.activation(qrs[:], qrow[:], AF.Tanh)
                psq = psA.tile([128, 4], F32, name="psq", tag="small", bufs=2)
                for ca in range(4):
                    nc.tensor.transpose(
                        psq[:, ca : ca + 1],
                        qrs[0:1, ca * 128 : (ca + 1) * 128],
                        t_id1[:],
                    )
                qT = scr.tile([128, 4], F32R, name="qT", tag="qT", bufs=2)
                nc.vector.tensor_copy(qT[:], psq[:])

                # e[t] = sum_a pre_enc[a,t] q[a]
                eps = psA.tile([1, T], F32, name="eps", tag="pe", bufs=2)
                for ca in range(4):
                    nc.tensor.matmul(
                        eps[:],
                        qT[:, ca : ca + 1],
                        t_pre[:, ca, :],
                        start=(ca == 0),
                        stop=(ca == 3),
                    )

                # gates0 psum: Whh0 part first (fills the softmax gap)
                g0 = psA.tile([1, 4, 512], F32, name="g0", tag="g", bufs=1)
                for jg in range(4):
                    sl = slice(jg * 512, (jg + 1) * 512)
                    for cd in range(4):
                        nc.tensor.matmul(
                            g0[:, jg, :],
                            z0T_prev[:, cd : cd + 1],
                            t_Whh0[:, cd, sl],
                            start=(cd == 0),
                            stop=False,
                        )

                # softmax row (on DVE/ACT, overlaps Whh0 stream)
                em = scr.tile([1, T], F32, name="em", tag="em")
                nc.vector.tensor_tensor(em[:], eps[:], t_mneg[:], ALU.add)
                ngmx = scr.tile([1, 1], F32, name="ngmx", tag="ngmx")
                nc.vector.tensor_reduce(
                    ngmx[:], em[:], AX.X, ALU.max, negate=True
                )
                wsc = scr.tile([1, T], F32, name="wsc", tag="wsc")
                sume = scr.tile([1, 1], F32, name="sume", tag="sume")
                nc.scalar.activation(
                    wsc[:], em[:], AF.Exp, bias=ngmx[:], accum_out=sume[:]
                )
                rinv = scr.tile([1, 1], F32, name="rinv", tag="rinv")
                nc.vector.reciprocal(rinv[:], sume[:])
                wrow = scr.tile([1, T], F32, name="wrow", tag="wrow")
                nc.vector.tensor_scalar_mul(wrow[:], wsc[:], rinv[:])
                wps = psA.tile([128, 2], F32, name="wps", tag="small", bufs=2)
                for ct in range(2):
                    nc.tensor.transpose(
                        wps[:, ct : ct + 1],
                        wrow[0:1, ct * 128 : (ct + 1) * 128],
                        t_id1[:],
                    )
                wT = scr.tile([128, 2], F32R, name="wT", tag="wT", bufs=2)
                nc.vector.tensor_copy(wT[:], wps[:])

                # rest of gates0: EYc one-hot inject + w@HW
                for jg in range(4):
                    sl = slice(jg * 512, (jg + 1) * 512)
                    nc.tensor.matmul(
                        g0[:, jg, :],
                        t_I64[:, u : u + 1],
                        t_EYc[:, jg, :],
                        start=False,
                        stop=False,
                    )
                    for ct in range(2):
                        nc.tensor.matmul(
                            g0[:, jg, :],
                            wT[:, ct : ct + 1],
                            t_HW[:, ct, sl],
                            start=False,
                            stop=(ct == 1),
                        )

                # pointwise cell 0: all-tanh form, hidden kept doubled
                th = [None] * 4
                for jg in range(4):
                    t = scr.tile([1, D], F32, name=f"th{jg}", tag=f"th{jg}")
                    nc.scalar.activation(
                        t[:], g0[:, jg, :], AF.Tanh, scale=SC[jg]
                    )
                    th[jg] = t
                tt1 = scr.tile([1, D], F32, name="tt1", tag="tt1")
                nc.vector.scalar_tensor_tensor(
                    tt1[:], th[1][:], 1.0, t_c0[:], ALU.add, ALU.mult
                )
                tt2 = scr.tile([1, D], F32, name="tt2", tag="tt2")
                nc.vector.scalar_tensor_tensor(
                    tt2[:], th[0][:], 1.0, th[2][:], ALU.add, ALU.mult
                )
                nc.vector.scalar_tensor_tensor(
                    t_c0[:], tt1[:], 0.5, tt2[:], ALU.mult, ALU.add
                )
                thc = scr.tile([1, D], F32, name="thc", tag="thc")
                nc.scalar.activation(thc[:], t_c0[:], AF.Tanh, scale=0.5)
                z0r = scr.tile([1, D], F32, name="z0r", tag="z0r")
                nc.vector.scalar_tensor_tensor(
                    z0r[:], th[3][:], 1.0, thc[:], ALU.add, ALU.mult
                )

                # z0T (doubled)
                zps = psA.tile([128, 4], F32, name="zps", tag="small", bufs=2)
                for cd in range(4):
                    nc.tensor.transpose(
                        zps[:, cd : cd + 1],
                        z0r[0:1, cd * 128 : (cd + 1) * 128],
                        t_id1[:],
                    )
                z0T = scr.tile([128, 4], F32R, name="z0T", tag="z0T", bufs=2)
                nc.vector.tensor_copy(z0T[:], zps[:])

                # gates1 = (Whh1/2)@Z1 + (Wih1/2)@Z0new   (+ b1 via DVE)
                g1 = psA.tile([1, 4, 512], F32, name="g1", tag="g", bufs=1)
                for jg in range(4):
                    sl = slice(jg * 512, (jg + 1) * 512)
                    for cd in range(4):
                        nc.tensor.matmul(
                            g1[:, jg, :],
                            z1T_prev[:, cd : cd + 1],
                            t_Whh1[:, cd, sl],
                            start=(cd == 0),
                            stop=False,
                        )
                    for cd in range(4):
                        nc.tensor.matmul(
                            g1[:, jg, :],
                            z0T[:, cd : cd + 1],
                            t_Wih1[:, cd, sl],
                            start=False,
                            stop=(cd == 3),
                        )

                # pointwise cell 1 (bias+scale fused into one DVE op per gate)
                th1 = [None] * 4
                for jg in range(4):
                    sl = slice(jg * 512, (jg + 1) * 512)
                    gin = scr.tile([1, D], F32, name=f"gin{jg}", tag="gin",
                                   bufs=2)
                    nc.vector.scalar_tensor_tensor(
                        gin[:], g1[:, jg, :], SC[jg], t_bih1[:, sl],
                        ALU.mult, ALU.add,
                    )
                    t = scr.tile([1, D], F32, name=f"sh{jg}", tag=f"th{jg}")
                    nc.scalar.activation(t[:], gin[:], AF.Tanh)
                    th1[jg] = t
                tt3 = scr.tile([1, D], F32, name="tt3", tag="tt1")
                nc.vector.scalar_tensor_tensor(
                    tt3[:], th1[1][:], 1.0, t_c1[:], ALU.add, ALU.mult
                )
                tt4 = scr.tile([1, D], F32, name="tt4", tag="tt2")
                nc.vector.scalar_tensor_tensor(
                    tt4[:], th1[0][:], 1.0, th1[2][:], ALU.add, ALU.mult
                )
                nc.vector.scalar_tensor_tensor(
                    t_c1[:], tt3[:], 0.5, tt4[:], ALU.mult, ALU.add
                )
                thc1 = scr.tile([1, D], F32, name="thc1", tag="thc")
                nc.scalar.activation(thc1[:], t_c1[:], AF.Tanh, scale=0.5)
                z1r = scr.tile([1, D], F32, name="z1r", tag="z0r")
                nc.vector.scalar_tensor_tensor(
                    z1r[:], th1[3][:], 1.0, thc1[:], ALU.add, ALU.mult
                )

                # z1T (doubled) -> Z1 column u
                zps1 = psA.tile([128, 4, 1], F32, name="zps1", tag="small",
                                bufs=2)
                for cd in range(4):
                    nc.tensor.transpose(
                        zps1[:, cd, :],
                        z1r[0:1, cd * 128 : (cd + 1) * 128],
                        t_id1[:],
                    )
                nc.vector.tensor_copy(t_Z1[:, :, u : u + 1], zps1[:])

                z0T_prev = z0T

        # ---------------- phase C: joint ----------------
        psAB.close()
        with tc.tile_pool(name="phC", bufs=1) as pC, \
             tc.tile_pool(name="psC", bufs=1, space="PSUM") as psC:
            t_Wld = pC.tile([128, 4, J], F32R, name="t_Wld", tag="t_Wld")
            t_Wout = pC.tile([128, 4, O], F32R, name="t_Wout", tag="t_Wout")
            t_bout = pC.tile([128, 4], F32, name="t_bout", tag="t_bout")
            nc.sync.dma_start(t_Wld[:], Wld_d[:])
            nc.sync.dma_start(t_Wout[:], Wout_d[:])
            nc.sync.dma_start(t_bout[:], bout_d[:])

            # DT[j, u] = sum_d (Wld/2)[d,j] Z1[u,d]
            t_DT = pC.tile([128, 4, U], F32, name="t_DT", tag="t_DT")
            for cj in range(4):
                dps = psC.tile([128, U], F32, name="dps", tag="j", bufs=4)
                for cd in range(4):
                    nc.tensor.matmul(
                        dps[:],
                        t_Wld[:, cd, cj * 128 : (cj + 1) * 128],
                        t_Z1[:, cd, :],
                        start=(cd == 0),
                        stop=(cd == 3),
                    )
                nc.vector.tensor_copy(t_DT[:, cj, :], dps[:])

            pair_us = [(u0, 2) for u0 in range(0, n_steps - 1, 2)]
            if n_steps % 2:
                pair_us.append((n_steps - 1, 1))
            for u0, kk in pair_us:
                zt = pC.tile([128, 4, kk, T], F32, name="zt", tag="zt", bufs=2)
                for cj in range(4):
                    for k in range(kk):
                        nc.vector.tensor_scalar(
                            zt[:, cj, k, :], t_aT[:, cj, :],
                            t_DT[:, cj, u0 + k : u0 + k + 1], None, ALU.add,
                        )
                zth = pC.tile([128, 4, kk, T], F32R, name="zth", tag="zth",
                              bufs=2)
                nc.scalar.activation(zth[:], zt[:], AF.Tanh)
                for m in range(4):
                    pj = psC.tile([OM, kk * T], F32, name="pj", tag="j", bufs=4)
                    for cj in range(4):
                        nc.tensor.matmul(
                            pj[:],
                            t_Wout[:, cj, m * OM : (m + 1) * OM],
                            zth[:, cj, :, :],
                            start=(cj == 0),
                            stop=(cj == 3),
                        )
                    outP = pC.tile([OM, kk, T], F32, name="outP", tag="outP",
                                   bufs=3)
                    nc.scalar.activation(
                        outP[:], pj[:], AF.Identity,
                        bias=t_bout[0:OM, m : m + 1],
                    )
                    nc.sync.dma_start(
                        out_d[m : m + 1, :, u0 : u0 + kk, :], outP[:]
                    )

    nc.compile()
    return nc


# ----------------------------------------------------------------------------
# entry point
# ----------------------------------------------------------------------------

def kernel(**inputs):
    global LAST_RESULTS
    if "nc" not in _CACHE:
        _CACHE["nc"] = _build(U)
    nc = _CACHE["nc"]
    in_maps = _prep_inputs(inputs)
    res = run_bass_kernel_spmd(
        nc, in_maps, list(range(NCORES)),
        trace=bool(int(os.environ.get("KBENCH_TRACE", "0"))),
    )
    LAST_RESULTS = res
    outs = []
    for c in range(NCORES):
        o = res.results[c]["out"]              # [4, 125, U, T]
        o = o.reshape(O, U, T).transpose(2, 1, 0)  # [T, U, O]
        outs.append(np.ascontiguousarray(o))
    full = np.stack(outs, axis=0).astype(np.float32)  # [B, T, U, O]
    return full
